# revision 27
# baseline (speedup 1.0000x reference)
"""Trainium2 Bass kernel for nn_Decoder (gnn_message_passing).

Self-contained: takes FULL unsharded inputs, shards 32 graph items across
8 NeuronCores (4 items/core, data-parallel), runs one Bass/Tile program
per core via run_bass_kernel_spmd, reassembles full outputs on host.

Reformulation (validated vs reference at ~1e-7 in numpy):
  - embedding gather      -> onehot matmul
  - relational msg pass   -> dense per-item adjacency matmuls:
                               msg^T = sum_r w_rel[r]^T (y^T A_r^T)   (blocks 1,2)
                               msg^T = sum_r (y w_rel[r])^T A_r^T     (block 3)
  - edge-score gather     -> incidence matmul: scores^T = uw^T Gcat^T
Host does only index preprocessing (build A/Gcat/onehot from int edge
lists), sharding/layout transforms, and unsharding. All float math runs
on device.
"""

import numpy as np
import ml_dtypes

import concourse.bass as bass
import concourse.bacc as bacc
import concourse.mybir as mybir
import concourse.tile as tile
from concourse.bass_utils import run_bass_kernel_spmd

# problem dims (hardcoded per contract)
B, SY, SX = 32, 512, 1024
F, IN, H, EMB, V = 512, 256, 512, 256, 128
NSLT, NSRT = 4, 5
NCORES = 8
BPC = B // NCORES            # items per core = 4
NYC = BPC * SY               # decoder nodes per core = 2048
NXC = BPC * SX               # encoder nodes per core = 4096
EPI = 3 * SY                 # edges per item = 1536
EPC = BPC * EPI              # edges per core = 6144

F32 = mybir.dt.float32
F32R = mybir.dt.float32r
BF16 = mybir.dt.bfloat16
AX = mybir.AxisListType.X
AF = mybir.ActivationFunctionType

BF = ml_dtypes.bfloat16

_CACHE = {}


def _r(ap):
    """reinterpret fp32 AP as float32r for full-rate PE"""
    return ap.bitcast(F32R)


def _build_program(limit=None):
    """Build the per-core Bass program (shared across all 8 cores).
    limit: None=full, 'init', 'b1', 'b2', 'b3' (for HW bisection)."""
    nc = bacc.Bacc("TRN2", target_bir_lowering=False, debug=False,
                   enable_asserts=False, num_devices=NCORES)

    def din(name, shape, dt=F32):
        return nc.dram_tensor(name, list(shape), dt, kind="ExternalInput").ap()

    def dout(name, shape, dt=F32):
        return nc.dram_tensor(name, list(shape), dt, kind="ExternalOutput").ap()

    # ---- DRAM inputs (per core) ----
    xT_h = din("xT", (F, NXC), BF16)                 # x shard, transposed, bf16
    A_h = din("A_T", (BPC, NSLT, SY, SY), BF16)      # A_r^T [item,r,src,tgt]
    G_h = din("GcatT", (BPC, 2 * SY, EPI), BF16)     # [item, 2*SY, EPI]
    oh_h = din("onehotT", (V, NYC), F32R)
    emb_h = din("embed", (V, IN), F32R)
    w_h = {}
    for blk, i_, o_ in ((1, IN, H), (2, H, H), (3, H, EMB)):
        w_h[f"w{blk}_self"] = din(f"w{blk}_self", (i_, o_), F32R)
        w_h[f"w{blk}_rel"] = din(f"w{blk}_rel", (NSLT, i_, o_),
                                 F32R if blk == 3 else BF16)
        if blk > 1:
            w_h[f"w{blk}_init"] = din(f"w{blk}_init", (IN, o_), F32R)
        w_h[f"w{blk}_q"] = din(f"w{blk}_q", (o_, o_), F32R)
        w_h[f"w{blk}_k"] = din(f"w{blk}_k", (F, o_), BF16)
        w_h[f"w{blk}_v"] = din(f"w{blk}_v", (F, o_), BF16)
    wz_h = din("wz", (EMB, V), F32R)
    wg_h = din("wg", (2 * EMB, 8), F32R)   # NSRT padded to 8 (fp32r N restriction)
    id_h = din("ident", (128, 128), F32R)
    bz_h = din("bz", (V,))
    bg_h = din("bg", (NSRT,))

    # ---- DRAM outputs (per core) ----
    yT_o = dout("yT_out", (EMB, NYC), F32R)                # host transposes
    ysc_o = dout("yscoreT_out", (V, NYC))            # host transposes
    esc_o = dout("escoreT_out", (NSRT, EPC))         # host transposes
    a_o = [dout(f"a{b}_out", (BPC, SY, SX), F32R) for b in (1, 2, 3)]

    with tile.TileContext(nc) as tc:
        _trace(nc, tc, xT_h, A_h, G_h, oh_h, emb_h, w_h, wz_h, wg_h, bz_h,
               bg_h, id_h, yT_o, ysc_o, esc_o, a_o, limit)
    nc.compile()
    return nc


def _trace(nc, tc, xT_h, A_h, G_h, oh_h, emb_h, w_h, wz_h, wg_h, bz_h, bg_h,
           id_h, yT_o, ysc_o, esc_o, a_o, limit=None):
    import contextlib
    ctx = contextlib.ExitStack()
    with ctx:
        pers = ctx.enter_context(tc.tile_pool(name="pers", bufs=1))
        # psum pools
        ph = ctx.enter_context(tc.tile_pool(name="ph", bufs=2, space="PSUM"))
        pS = ctx.enter_context(tc.tile_pool(name="pS", bufs=2, space="PSUM"))
        pmm = ctx.enter_context(tc.tile_pool(name="pmm", bufs=2, space="PSUM"))
        ptr = ctx.enter_context(tc.tile_pool(name="ptr", bufs=2, space="PSUM"))
        pst = ctx.enter_context(tc.tile_pool(name="pst", bufs=4))
        # sbuf working pools (released before the heads phase)
        wctx = contextlib.ExitStack()
        pA = wctx.enter_context(tc.tile_pool(name="pA", bufs=1))
        pxT = wctx.enter_context(tc.tile_pool(name="pxT", bufs=1))
        phT = wctx.enter_context(tc.tile_pool(name="phT", bufs=2))
        pz = wctx.enter_context(tc.tile_pool(name="pz", bufs=1))
        pkv = wctx.enter_context(tc.tile_pool(name="pkv", bufs=1))
        pq = wctx.enter_context(tc.tile_pool(name="pq", bufs=1))
        pal = wctx.enter_context(tc.tile_pool(name="pal", bufs=2))
        palT = wctx.enter_context(tc.tile_pool(name="palT", bufs=1))
        wp = wctx.enter_context(tc.tile_pool(name="wp", bufs=1))

        sync = nc.sync

        # ---- persistent tiles ----
        yT = pers.tile([128, 4, NYC], F32R, tag="yT")        # [feat<=512, node]
        ynm = pers.tile([128, 16, 512], BF16, tag="ynm")    # [node, feat<=512]
        yinitT = pers.tile([128, 2, NYC], F32R, tag="yinitT")
        # onehotT is init-phase-only: borrow the A slot (A first used later;
        # WAR dep just orders block1's first A load after init consumes this)
        ohT = pA.tile([128, NYC], F32R, tag="A")
        emb_s = pers.tile([128, IN], F32R, tag="emb")
        ident = pers.tile([128, 128], F32R, tag="ident")
        bz_s = pers.tile([128, 1], F32, tag="bz")
        bg_s = pers.tile([8, 1], F32, tag="bg")

        sync.dma_start(out=ident, in_=id_h)
        sync.dma_start(out=ohT, in_=oh_h)
        sync.dma_start(out=emb_s, in_=emb_h)
        sync.dma_start(out=bz_s, in_=bz_h.rearrange("(p one) -> p one", one=1))
        nc.vector.memset(bg_s, 0.0)
        sync.dma_start(out=bg_s[:NSRT, :], in_=bg_h.rearrange("(p one) -> p one", one=1))

        # ---- y_init^T = embed^T @ onehotT ;  y_init (node-major) -> ynm ----
        for mt in range(2):
            for ns in range(4):
                ps = pmm.tile([128, 512], F32, tag="mm")
                nc.tensor.matmul(ps, _r(emb_s[:, mt * 128:(mt + 1) * 128]),
                                 _r(ohT[:, ns * 512:(ns + 1) * 512]))
                nc.scalar.copy(yinitT[:, mt, ns * 512:(ns + 1) * 512], ps)
        for nt in range(16):
            ps = pmm.tile([128, 512], F32, tag="mm")
            nc.tensor.matmul(ps[:, :IN],
                             _r(ohT[:, nt * 128:(nt + 1) * 128]), _r(emb_s))
            nc.vector.tensor_copy(ynm[:, nt, :IN], ps[:, :IN])

        inv_sqrt = {1: 1.0 / np.sqrt(512.0), 2: 1.0 / np.sqrt(512.0),
                    3: 1.0 / np.sqrt(256.0)}

        # ================= blocks =================
        nblk = {'init': 0, 'b1': 1, 'b2': 2, 'b3': 3}.get(limit, 3)
        for blk, IN_b, O_b in ((1, IN, H), (2, H, H), (3, H, EMB))[:nblk]:
            ikt = IN_b // 128     # input-feat K tiles
            omt = O_b // 128      # output-feat M tiles
            ws = wp.tile([128, ikt, O_b], F32R, tag="wself")
            sync.dma_start(out=ws, in_=w_h[f"w{blk}_self"].rearrange(
                "(kt p) o -> p kt o", p=128))
            rel_dt = F32R if blk == 3 else BF16
            wr = wp.tile([128, ikt, NSLT, O_b], rel_dt, tag="wrel")
            for r in range(NSLT):
                sync.dma_start(out=wr[:, :, r, :],
                               in_=w_h[f"w{blk}_rel"][r].rearrange(
                                   "(kt p) o -> p kt o", p=128))
            if blk > 1:
                wi = wp.tile([128, 2, O_b], F32R, tag="winit")
                sync.dma_start(out=wi, in_=w_h[f"w{blk}_init"].rearrange(
                    "(kt p) o -> p kt o", p=128))
            wq = wp.tile([128, omt, O_b], F32R, tag="wq")
            sync.dma_start(out=wq, in_=w_h[f"w{blk}_q"].rearrange(
                "(kt p) o -> p kt o", p=128))
            wk = wp.tile([128, 4, O_b], BF16, tag="wk")
            sync.dma_start(out=wk, in_=w_h[f"w{blk}_k"].rearrange(
                "(kt p) o -> p kt o", p=128))
            wv = wp.tile([128, 4, O_b], BF16, tag="wv")
            sync.dma_start(out=wv, in_=w_h[f"w{blk}_v"].rearrange(
                "(kt p) o -> p kt o", p=128))

            for it in range(BPC):
                _item(nc, tc, blk, it, IN_b, O_b, ikt, omt, inv_sqrt[blk],
                      ws, wr, wi if blk > 1 else None, wq, wk, wv,
                      yT, ynm, yinitT, ident,
                      xT_h, A_h, a_o[blk - 1],
                      ph, pS, pmm, ptr, pA, pxT, phT, pz, pkv, pq, pal,
                      palT, pst, sync)

        # ================= heads =================
        if limit in ('init', 'b1', 'b2', 'b3'):
            wctx.close()
            return
        wctx.close()  # release per-item working pools -> room for heads
        phd = ctx.enter_context(tc.tile_pool(name="phd", bufs=2))
        # y_score^T = wz^T y3^T + bz
        wz_s = phd.tile([128, 2, V], F32R, tag="wz")
        sync.dma_start(out=wz_s, in_=wz_h.rearrange("(kt p) o -> p kt o", p=128))
        wg_s = phd.tile([128, 4, 8], F32R, tag="wg")
        sync.dma_start(out=wg_s, in_=wg_h.rearrange("(kt p) o -> p kt o", p=128))
        for ns in range(4):
            ps = pmm.tile([128, 512], F32, tag="mm")
            for kt in range(2):
                nc.tensor.matmul(ps, _r(wz_s[:, kt, :]),
                                 _r(yT[:, kt, ns * 512:(ns + 1) * 512]),
                                 start=kt == 0, stop=kt == 1)
            ysc = phd.tile([128, 512], F32, tag="ysc")
            nc.scalar.activation(ysc, ps, AF.Identity, bias=bz_s)
            sync.dma_start(out=ysc_o[:, ns * 512:(ns + 1) * 512], in_=ysc)
        if limit == 'ysc':
            return
        # edge scores
        for it in range(BPC):
            uw = phd.tile([128, 8, 8], BF16, tag="uw")
            for side in range(2):
                for nmt in range(4):
                    ps = pmm.tile([128, 8], F32, tag="mm")
                    for kt in range(2):
                        nc.tensor.matmul(
                            ps,
                            _r(yT[:, kt, it * SY + nmt * 128:
                                  it * SY + (nmt + 1) * 128]),
                            _r(wg_s[:, side * 2 + kt, :]),
                            start=kt == 0, stop=kt == 1)
                    nc.vector.tensor_copy(uw[:, side * 4 + nmt, :], ps)
            esc = phd.tile([8, EPI], F32, tag="esc")
            for es in range(3):
                gc = phd.tile([128, 8, 512], BF16, tag="gc", bufs=8)
                sync.dma_start(out=gc, in_=G_h[it][:, es * 512:(es + 1) * 512]
                               .rearrange("(kt p) e -> p kt e", p=128))
                ps = pmm.tile([8, 512], F32, tag="mm")
                for kc in range(8):
                    nc.tensor.matmul(ps, uw[:, kc, :], gc[:, kc, :],
                                     start=kc == 0, stop=kc == 7)
                nc.scalar.activation(esc[:, es * 512:(es + 1) * 512], ps,
                                     AF.Identity, bias=bg_s)
            sync.dma_start(out=esc_o[:, it * EPI:(it + 1) * EPI], in_=esc[:NSRT, :])
        if limit == 'esc':
            return
        # final y (block3 output, feature-major; host transposes)
        sync.dma_start(out=yT_o.rearrange("(kt p) n -> p kt n", p=128),
                       in_=yT[:, 0:2, :])


def _item(nc, tc, blk, it, IN_b, O_b, ikt, omt, isq,
          ws, wr, wi, wq, wk, wv, yT, ynm, yinitT, ident,
          xT_h, A_h, a_out,
          ph, pS, pmm, ptr, pA, pxT, phT, pz, pkv, pq, pal, palT, pst, sync):
    """One graph item within one block."""
    c0 = it * SY                  # node col offset of this item
    sl = slice(c0, c0 + SY)

    # y^T source for this block (block1 reads y_init^T)
    def yT_in(kt, cols):
        if blk == 1:
            return yinitT[:, kt, cols]
        return yT[:, kt, cols]

    # ---- load A for this item ----
    A_s = pA.tile([128, 4, NSLT, SY], BF16, tag="A")
    for r in range(NSLT):
        sync.dma_start(out=A_s[:, :, r, :], in_=A_h[it, r].rearrange(
            "(st p) t -> p st t", p=128))

    # ---- message-passing operand ----
    if blk < 3:
        # z_r^T = y^T A_r^T  for all r -> z_s [feat, r, tgt] bf16
        z_s = pz.tile([128, NSLT, ikt, SY], BF16, tag="z")
        for r in range(NSLT):
            for zmt in range(ikt):
                ps = pmm.tile([128, 512], F32, tag="mm")
                for skt in range(4):
                    nc.tensor.matmul(
                        ps,
                        ynm[:, it * 4 + skt, zmt * 128:(zmt + 1) * 128],
                        A_s[:, skt, r, :],
                        start=skt == 0, stop=skt == 3)
                nc.vector.tensor_copy(z_s[:, r, zmt, :], ps)
    else:
        # p_r = y_b @ w_rel[r] (node-major) -> p_s [node, r, feat] bf16
        p_s = pz.tile([128, NSLT, 4, O_b], BF16, tag="z")
        for r in range(NSLT):
            for nmt in range(4):
                ps = pmm.tile([128, O_b], F32, tag="mm")
                for kt in range(ikt):
                    nc.tensor.matmul(
                        ps,
                        _r(yT_in(kt, slice(c0 + nmt * 128, c0 + (nmt + 1) * 128))),
                        _r(wr[:, kt, r, :]),
                        start=kt == 0, stop=kt == ikt - 1)
                nc.vector.tensor_copy(p_s[:, r, nmt, :], ps)

    # ---- h^T accumulation per output M-tile ----
    hT = phT.tile([128, 4, SY], F32R, tag="hT")
    for mt in range(omt):
        mts = slice(mt * 128, (mt + 1) * 128)
        hp = ph.tile([128, SY], F32, tag="h")
        n_mm = ikt + (2 if wi is not None else 0) + \
            (NSLT * ikt if blk < 3 else NSLT * 4)
        i_mm = 0
        for kt in range(ikt):
            nc.tensor.matmul(hp, _r(ws[:, kt, mts]), _r(yT_in(kt, sl)),
                             start=i_mm == 0, stop=i_mm == n_mm - 1)
            i_mm += 1
        if wi is not None:
            for kt in range(2):
                nc.tensor.matmul(hp, _r(wi[:, kt, mts]), _r(yinitT[:, kt, sl]),
                                 start=i_mm == 0, stop=i_mm == n_mm - 1)
                i_mm += 1
        if blk < 3:
            for r in range(NSLT):
                for kt in range(ikt):
                    nc.tensor.matmul(hp, wr[:, kt, r, mts], z_s[:, r, kt, :],
                                     start=i_mm == 0, stop=i_mm == n_mm - 1)
                    i_mm += 1
        else:
            for r in range(NSLT):
                for skt in range(4):
                    nc.tensor.matmul(hp, p_s[:, r, skt, mts], A_s[:, skt, r, :],
                                     start=i_mm == 0, stop=i_mm == n_mm - 1)
                    i_mm += 1
        nc.scalar.copy(hT[:, mt, :], hp)

    # ---- attention ----
    xT_s = pxT.tile([128, 4, SX], BF16, tag="xT")
    sync.dma_start(out=xT_s, in_=xT_h[:, it * SX:(it + 1) * SX].rearrange(
        "(kt p) n -> p kt n", p=128))

    qT = pq.tile([128, 4, SY], BF16, tag="qT")
    for mt in range(omt):
        ps = pmm.tile([128, SY], F32, tag="mm")
        for kt in range(omt):
            nc.tensor.matmul(ps, _r(wq[:, kt, mt * 128:(mt + 1) * 128]),
                             _r(hT[:, kt, :]), start=kt == 0, stop=kt == omt - 1)
        nc.scalar.copy(qT[:, mt, :], ps)

    kT = pkv.tile([128, 4, SX], BF16, tag="kT")
    for mt in range(omt):
        for ns in range(2):
            ps = pmm.tile([128, 512], F32, tag="mm")
            for kt in range(4):
                nc.tensor.matmul(ps, wk[:, kt, mt * 128:(mt + 1) * 128],
                                 xT_s[:, kt, ns * 512:(ns + 1) * 512],
                                 start=kt == 0, stop=kt == 3)
            nc.scalar.copy(kT[:, mt, ns * 512:(ns + 1) * 512], ps)

    v_s = pkv.tile([128, 8, O_b], BF16, tag="v")
    for nmt in range(8):
        ps = pmm.tile([128, O_b], F32, tag="mm")
        for kt in range(4):
            nc.tensor.matmul(ps, xT_s[:, kt, nmt * 128:(nmt + 1) * 128],
                             wv[:, kt, :], start=kt == 0, stop=kt == 3)
        nc.vector.tensor_copy(v_s[:, nmt, :], ps)

    alT = palT.tile([128, 8, SY], BF16, tag="alT")
    for qt in range(4):
        # S in two half-tiles so exp(half) overlaps next half's matmuls
        Sh = []
        mxs = []
        for ns in range(2):
            Sp = pS.tile([128, 512], F32, tag="S")
            for kt in range(omt):
                nc.tensor.matmul(Sp,
                                 qT[:, kt, qt * 128:(qt + 1) * 128],
                                 kT[:, kt, ns * 512:(ns + 1) * 512],
                                 start=kt == 0, stop=kt == omt - 1)
            m = pst.tile([128, 1], F32, tag=f"mx{ns}")
            nc.vector.reduce_max(m, Sp, axis=AX)
            Sh.append(Sp)
            mxs.append(m)
        mx = pst.tile([128, 1], F32, tag="mx")
        nc.vector.tensor_max(mx, mxs[0], mxs[1])
        nm = pst.tile([128, 1], F32, tag="nm")
        nc.vector.tensor_scalar_mul(nm, mx, -isq)
        al = pal.tile([128, SX], F32R, tag="al")
        sums = []
        for ns in range(2):
            s = pst.tile([128, 1], F32, tag=f"s{ns}")
            nc.scalar.activation(al[:, ns * 512:(ns + 1) * 512], Sh[ns],
                                 AF.Exp, bias=nm, scale=isq, accum_out=s)
            sums.append(s)
        ssum = pst.tile([128, 1], F32, tag="ssum")
        nc.vector.tensor_add(ssum, sums[0], sums[1])
        rs = pst.tile([128, 1], F32, tag="rs")
        nc.vector.reciprocal(rs, ssum)
        nc.vector.tensor_scalar_mul(al, al, rs)
        sync.dma_start(out=a_out[it, qt * 128:(qt + 1) * 128, :], in_=al)
        for kc in range(8):
            tp = ptr.tile([128, 128], F32R, tag="tr")
            nc.tensor.transpose(tp, _r(al[:, kc * 128:(kc + 1) * 128]),
                                _r(ident))
            if kc % 2 == 0:
                nc.vector.tensor_copy(alT[:, kc, qt * 128:(qt + 1) * 128],
                                      tp)
            else:
                nc.scalar.copy(alT[:, kc, qt * 128:(qt + 1) * 128],
                               tp)

    # ctx^T accumulate + y^T = relu(h^T + ctx^T)
    for mt in range(omt):
        cp = ph.tile([128, SY], F32, tag="h")
        for kc in range(8):
            nc.tensor.matmul(cp, v_s[:, kc, mt * 128:(mt + 1) * 128],
                             alT[:, kc, :], start=kc == 0, stop=kc == 7)
        ycols = yT[:, mt, sl]
        nc.vector.tensor_add(ycols, hT[:, mt, :], cp)
        nc.scalar.activation(ycols, ycols, AF.Relu)

    # y node-major for next block's z matmuls (blocks 1,2 only)
    if blk < 3:
        for nt in range(4):
            for fmt in range(omt):
                tp = ptr.tile([128, 128], F32R, tag="tr")
                nc.tensor.transpose(
                    tp, _r(yT[:, fmt, c0 + nt * 128:c0 + (nt + 1) * 128]),
                    _r(ident))
                if fmt % 2 == 0:
                    nc.vector.tensor_copy(
                        ynm[:, it * 4 + nt, fmt * 128:(fmt + 1) * 128],
                        tp)
                else:
                    nc.scalar.copy(
                        ynm[:, it * 4 + nt, fmt * 128:(fmt + 1) * 128],
                        tp)


# ======================= host side =======================

def _host_prep(inputs):
    src_g = np.asarray(inputs["tgt_edge_index"][0]).astype(np.int64)
    tgt_g = np.asarray(inputs["tgt_edge_index"][1]).astype(np.int64)
    et = np.asarray(inputs["tgt_edge_type"]).astype(np.int64)
    tgt_y = np.asarray(inputs["tgt_y"]).astype(np.int64)
    x = np.ascontiguousarray(np.asarray(inputs["x"], dtype=np.float32))

    weights = {}
    for blk in (1, 2, 3):
        weights[f"w{blk}_self"] = np.ascontiguousarray(inputs[f"w{blk}_self"], np.float32)
        rel = np.ascontiguousarray(inputs[f"w{blk}_rel"], np.float32)
        weights[f"w{blk}_rel"] = rel if blk == 3 else rel.astype(BF)
        if blk > 1:
            weights[f"w{blk}_init"] = np.ascontiguousarray(inputs[f"w{blk}_init"], np.float32)
        weights[f"w{blk}_q"] = np.ascontiguousarray(inputs[f"w{blk}_q"], np.float32)
        weights[f"w{blk}_k"] = np.ascontiguousarray(inputs[f"w{blk}_k"], np.float32).astype(BF)
        weights[f"w{blk}_v"] = np.ascontiguousarray(inputs[f"w{blk}_v"], np.float32).astype(BF)
    weights["wz"] = np.ascontiguousarray(inputs["wz"], np.float32)
    wg8 = np.zeros((2 * EMB, 8), np.float32)
    wg8[:, :NSRT] = np.asarray(inputs["wg"], np.float32)
    weights["wg"] = wg8
    weights["bz"] = np.ascontiguousarray(inputs["bz"], np.float32)
    weights["bg"] = np.ascontiguousarray(inputs["bg"], np.float32)
    weights["embed"] = np.ascontiguousarray(inputs["embed"], np.float32)

    in_maps = []
    ar = np.arange(EPI)
    for c in range(NCORES):
        A_T = np.zeros((BPC, NSLT, SY, SY), np.float32)
        GcatT = np.zeros((BPC, 2 * SY, EPI), np.float32)
        for i in range(BPC):
            b = c * BPC + i
            e0 = b * EPI
            s = src_g[e0:e0 + EPI] - b * SY
            t = tgt_g[e0:e0 + EPI] - b * SY
            r = et[e0:e0 + EPI]
            np.add.at(A_T[i], (r, s, t), 1.0)
            GcatT[i][s, ar] += 1.0
            GcatT[i][SY + t, ar] += 1.0
        ty = tgt_y[c * NYC:(c + 1) * NYC]
        onehotT = np.zeros((V, NYC), np.float32)
        onehotT[ty, np.arange(NYC)] = 1.0
        xT = np.ascontiguousarray(x[c * NXC:(c + 1) * NXC].T).astype(BF)
        m = dict(weights)
        m["xT"] = xT
        m["A_T"] = A_T.astype(BF)
        m["GcatT"] = GcatT.astype(BF)
        m["onehotT"] = onehotT
        m["ident"] = np.eye(128, dtype=np.float32)
        in_maps.append(m)
    return in_maps


def _get_nc(limit=None):
    key = ("nc", limit)
    if key not in _CACHE:
        _CACHE[key] = _build_program(limit)
    return _CACHE[key]


def kernel(**inputs):
    nc = _get_nc(_CACHE.get('limit'))
    in_maps = _host_prep(inputs)
    res = run_bass_kernel_spmd(nc, in_maps, core_ids=list(range(NCORES)),
                               **_CACHE.get("run_kwargs", {}))
    _CACHE["last_results"] = res
    rs = res.results
    y = np.concatenate([np.ascontiguousarray(r["yT_out"].T) for r in rs], 0)
    y_score = np.concatenate([np.ascontiguousarray(r["yscoreT_out"].T) for r in rs], 0)
    y_edge = np.concatenate([np.ascontiguousarray(r["escoreT_out"].T) for r in rs], 0)
    a1 = np.concatenate([r["a1_out"] for r in rs], 0)
    a2 = np.concatenate([r["a2_out"] for r in rs], 0)
    a3 = np.concatenate([r["a3_out"] for r in rs], 0)
    return (y, inputs["tgt_y_batch"], inputs["tgt_edge_index"],
            inputs["tgt_edge_type"], y_score, y_edge, a1, a2, a3)


# revision 29
# speedup vs baseline: 1.0023x; 1.0023x over previous
"""Trainium2 Bass kernel for nn_Decoder (gnn_message_passing).

Self-contained: takes FULL unsharded inputs, shards 32 graph items across
8 NeuronCores (4 items/core, data-parallel), runs one Bass/Tile program
per core via run_bass_kernel_spmd, reassembles full outputs on host.

Reformulation (validated vs reference at ~1e-7 in numpy):
  - embedding gather      -> onehot matmul
  - relational msg pass   -> dense per-item adjacency matmuls:
                               msg^T = sum_r w_rel[r]^T (y^T A_r^T)   (blocks 1,2)
                               msg^T = sum_r (y w_rel[r])^T A_r^T     (block 3)
  - edge-score gather     -> incidence matmul: scores^T = uw^T Gcat^T
Host does only index preprocessing (build A/Gcat/onehot from int edge
lists), sharding/layout transforms, and unsharding. All float math runs
on device.
"""

import numpy as np
import ml_dtypes

import concourse.bass as bass
import concourse.bacc as bacc
import concourse.mybir as mybir
import concourse.tile as tile
from concourse.bass_utils import run_bass_kernel_spmd

# problem dims (hardcoded per contract)
B, SY, SX = 32, 512, 1024
F, IN, H, EMB, V = 512, 256, 512, 256, 128
NSLT, NSRT = 4, 5
NCORES = 8
BPC = B // NCORES            # items per core = 4
NYC = BPC * SY               # decoder nodes per core = 2048
NXC = BPC * SX               # encoder nodes per core = 4096
EPI = 3 * SY                 # edges per item = 1536
EPC = BPC * EPI              # edges per core = 6144

F32 = mybir.dt.float32
F32R = mybir.dt.float32r
BF16 = mybir.dt.bfloat16
AX = mybir.AxisListType.X
AF = mybir.ActivationFunctionType

BF = ml_dtypes.bfloat16

_CACHE = {}


def _r(ap):
    """reinterpret fp32 AP as float32r for full-rate PE"""
    return ap.bitcast(F32R)


def _build_program(limit=None):
    """Build the per-core Bass program (shared across all 8 cores).
    limit: None=full, 'init', 'b1', 'b2', 'b3' (for HW bisection)."""
    nc = bacc.Bacc("TRN2", target_bir_lowering=False, debug=False,
                   enable_asserts=False, num_devices=NCORES)

    def din(name, shape, dt=F32):
        return nc.dram_tensor(name, list(shape), dt, kind="ExternalInput").ap()

    def dout(name, shape, dt=F32):
        return nc.dram_tensor(name, list(shape), dt, kind="ExternalOutput").ap()

    # ---- DRAM inputs (per core) ----
    xT_h = din("xT", (F, NXC), BF16)                 # x shard, transposed, bf16
    A_h = din("A_T", (BPC, NSLT, SY, SY), BF16)      # A_r^T [item,r,src,tgt]
    G_h = din("GcatT", (BPC, 2 * SY, EPI), BF16)     # [item, 2*SY, EPI]
    oh_h = din("onehotT", (V, NYC), F32R)
    emb_h = din("embed", (V, IN), F32R)
    w_h = {}
    for blk, i_, o_ in ((1, IN, H), (2, H, H), (3, H, EMB)):
        w_h[f"w{blk}_self"] = din(f"w{blk}_self", (i_, o_), F32R)
        w_h[f"w{blk}_rel"] = din(f"w{blk}_rel", (NSLT, i_, o_),
                                 F32R if blk == 3 else BF16)
        if blk > 1:
            w_h[f"w{blk}_init"] = din(f"w{blk}_init", (IN, o_), F32R)
        w_h[f"w{blk}_q"] = din(f"w{blk}_q", (o_, o_), F32R)
        w_h[f"w{blk}_k"] = din(f"w{blk}_k", (F, o_), BF16)
        w_h[f"w{blk}_v"] = din(f"w{blk}_v", (F, o_), BF16)
    wz_h = din("wz", (EMB, V), F32R)
    wg_h = din("wg", (2 * EMB, 8), F32R)   # NSRT padded to 8 (fp32r N restriction)
    id_h = din("ident", (128, 128), F32R)
    bz_h = din("bz", (V,))
    bg_h = din("bg", (NSRT,))

    # ---- DRAM outputs (per core) ----
    yT_o = dout("yT_out", (EMB, NYC), F32R)                # host transposes
    ysc_o = dout("yscoreT_out", (V, NYC))            # host transposes
    esc_o = dout("escoreT_out", (NSRT, EPC))         # host transposes
    a_o = [dout(f"a{b}_out", (BPC, SY, SX), F32R) for b in (1, 2, 3)]

    with tile.TileContext(nc) as tc:
        _trace(nc, tc, xT_h, A_h, G_h, oh_h, emb_h, w_h, wz_h, wg_h, bz_h,
               bg_h, id_h, yT_o, ysc_o, esc_o, a_o, limit)
    nc.compile()
    return nc


def _trace(nc, tc, xT_h, A_h, G_h, oh_h, emb_h, w_h, wz_h, wg_h, bz_h, bg_h,
           id_h, yT_o, ysc_o, esc_o, a_o, limit=None):
    import contextlib
    ctx = contextlib.ExitStack()
    with ctx:
        pers = ctx.enter_context(tc.tile_pool(name="pers", bufs=1))
        # psum pools
        ph = ctx.enter_context(tc.tile_pool(name="ph", bufs=2, space="PSUM"))
        pS = ctx.enter_context(tc.tile_pool(name="pS", bufs=2, space="PSUM"))
        pmm = ctx.enter_context(tc.tile_pool(name="pmm", bufs=2, space="PSUM"))
        ptr = ctx.enter_context(tc.tile_pool(name="ptr", bufs=2, space="PSUM"))
        pst = ctx.enter_context(tc.tile_pool(name="pst", bufs=4))
        # sbuf working pools (released before the heads phase)
        wctx = contextlib.ExitStack()
        pA = wctx.enter_context(tc.tile_pool(name="pA", bufs=1))
        pxT = wctx.enter_context(tc.tile_pool(name="pxT", bufs=1))
        phT = wctx.enter_context(tc.tile_pool(name="phT", bufs=2))
        pz = wctx.enter_context(tc.tile_pool(name="pz", bufs=1))
        pkv = wctx.enter_context(tc.tile_pool(name="pkv", bufs=1))
        pq = wctx.enter_context(tc.tile_pool(name="pq", bufs=1))
        pal = wctx.enter_context(tc.tile_pool(name="pal", bufs=2))
        palT = wctx.enter_context(tc.tile_pool(name="palT", bufs=1))
        wp = wctx.enter_context(tc.tile_pool(name="wp", bufs=1))

        sync = nc.sync

        # ---- persistent tiles ----
        yT = pers.tile([128, 4, NYC], F32R, tag="yT")        # [feat<=512, node]
        ynm = pers.tile([128, 16, 512], BF16, tag="ynm")    # [node, feat<=512]
        yinitT = pers.tile([128, 2, NYC], F32R, tag="yinitT")
        # onehotT is init-phase-only: borrow the A slot (A first used later;
        # WAR dep just orders block1's first A load after init consumes this)
        ohT = pA.tile([128, NYC], F32R, tag="A")
        emb_s = pers.tile([128, IN], F32R, tag="emb")
        ident = pers.tile([128, 128], F32R, tag="ident")
        bz_s = pers.tile([128, 1], F32, tag="bz")
        bg_s = pers.tile([8, 1], F32, tag="bg")

        sync.dma_start(out=ident, in_=id_h)
        sync.dma_start(out=ohT, in_=oh_h)
        sync.dma_start(out=emb_s, in_=emb_h)
        sync.dma_start(out=bz_s, in_=bz_h.rearrange("(p one) -> p one", one=1))
        nc.vector.memset(bg_s, 0.0)
        sync.dma_start(out=bg_s[:NSRT, :], in_=bg_h.rearrange("(p one) -> p one", one=1))

        # ---- y_init^T = embed^T @ onehotT ;  y_init (node-major) -> ynm ----
        for mt in range(2):
            for ns in range(4):
                ps = pmm.tile([128, 512], F32, tag="mm")
                nc.tensor.matmul(ps, _r(emb_s[:, mt * 128:(mt + 1) * 128]),
                                 _r(ohT[:, ns * 512:(ns + 1) * 512]))
                nc.scalar.copy(yinitT[:, mt, ns * 512:(ns + 1) * 512], ps)
        for nt in range(16):
            ps = pmm.tile([128, 512], F32, tag="mm")
            nc.tensor.matmul(ps[:, :IN],
                             _r(ohT[:, nt * 128:(nt + 1) * 128]), _r(emb_s))
            nc.vector.tensor_copy(ynm[:, nt, :IN], ps[:, :IN])

        inv_sqrt = {1: 1.0 / np.sqrt(512.0), 2: 1.0 / np.sqrt(512.0),
                    3: 1.0 / np.sqrt(256.0)}

        # ================= blocks =================
        nblk = {'init': 0, 'b1': 1, 'b2': 2, 'b3': 3}.get(limit, 3)
        for blk, IN_b, O_b in ((1, IN, H), (2, H, H), (3, H, EMB))[:nblk]:
            ikt = IN_b // 128     # input-feat K tiles
            omt = O_b // 128      # output-feat M tiles
            ws = wp.tile([128, ikt, O_b], F32R, tag="wself")
            sync.dma_start(out=ws, in_=w_h[f"w{blk}_self"].rearrange(
                "(kt p) o -> p kt o", p=128))
            rel_dt = F32R if blk == 3 else BF16
            wr = wp.tile([128, ikt, NSLT, O_b], rel_dt, tag="wrel")
            for r in range(NSLT):
                sync.dma_start(out=wr[:, :, r, :],
                               in_=w_h[f"w{blk}_rel"][r].rearrange(
                                   "(kt p) o -> p kt o", p=128))
            if blk > 1:
                wi = wp.tile([128, 2, O_b], F32R, tag="winit")
                sync.dma_start(out=wi, in_=w_h[f"w{blk}_init"].rearrange(
                    "(kt p) o -> p kt o", p=128))
            wq = wp.tile([128, omt, O_b], F32R, tag="wq")
            sync.dma_start(out=wq, in_=w_h[f"w{blk}_q"].rearrange(
                "(kt p) o -> p kt o", p=128))
            wk = wp.tile([128, 4, O_b], BF16, tag="wk")
            sync.dma_start(out=wk, in_=w_h[f"w{blk}_k"].rearrange(
                "(kt p) o -> p kt o", p=128))
            wv = wp.tile([128, 4, O_b], BF16, tag="wv")
            sync.dma_start(out=wv, in_=w_h[f"w{blk}_v"].rearrange(
                "(kt p) o -> p kt o", p=128))

            for it in range(BPC):
                _item(nc, tc, blk, it, IN_b, O_b, ikt, omt, inv_sqrt[blk],
                      ws, wr, wi if blk > 1 else None, wq, wk, wv,
                      yT, ynm, yinitT, ident,
                      xT_h, A_h, a_o[blk - 1],
                      ph, pS, pmm, ptr, pA, pxT, phT, pz, pkv, pq, pal,
                      palT, pst, sync)

        # ================= heads =================
        if limit in ('init', 'b1', 'b2', 'b3'):
            wctx.close()
            return
        wctx.close()  # release per-item working pools -> room for heads
        phd = ctx.enter_context(tc.tile_pool(name="phd", bufs=2))
        # y_score^T = wz^T y3^T + bz
        wz_s = phd.tile([128, 2, V], F32R, tag="wz")
        sync.dma_start(out=wz_s, in_=wz_h.rearrange("(kt p) o -> p kt o", p=128))
        wg_s = phd.tile([128, 4, 8], F32R, tag="wg")
        sync.dma_start(out=wg_s, in_=wg_h.rearrange("(kt p) o -> p kt o", p=128))
        for ns in range(4):
            ps = pmm.tile([128, 512], F32, tag="mm")
            for kt in range(2):
                nc.tensor.matmul(ps, _r(wz_s[:, kt, :]),
                                 _r(yT[:, kt, ns * 512:(ns + 1) * 512]),
                                 start=kt == 0, stop=kt == 1)
            ysc = phd.tile([128, 512], F32, tag="ysc")
            nc.scalar.activation(ysc, ps, AF.Identity, bias=bz_s)
            sync.dma_start(out=ysc_o[:, ns * 512:(ns + 1) * 512], in_=ysc)
        if limit == 'ysc':
            return
        # edge scores
        for it in range(BPC):
            uw = phd.tile([128, 8, 8], BF16, tag="uw")
            for side in range(2):
                for nmt in range(4):
                    ps = pmm.tile([128, 8], F32, tag="mm")
                    for kt in range(2):
                        nc.tensor.matmul(
                            ps,
                            _r(yT[:, kt, it * SY + nmt * 128:
                                  it * SY + (nmt + 1) * 128]),
                            _r(wg_s[:, side * 2 + kt, :]),
                            start=kt == 0, stop=kt == 1)
                    nc.vector.tensor_copy(uw[:, side * 4 + nmt, :], ps)
            esc = phd.tile([8, EPI], F32, tag="esc")
            for es in range(3):
                gc = phd.tile([128, 8, 512], BF16, tag="gc", bufs=8)
                sync.dma_start(out=gc, in_=G_h[it][:, es * 512:(es + 1) * 512]
                               .rearrange("(kt p) e -> p kt e", p=128))
                ps = pmm.tile([8, 512], F32, tag="mm")
                for kc in range(8):
                    nc.tensor.matmul(ps, uw[:, kc, :], gc[:, kc, :],
                                     start=kc == 0, stop=kc == 7)
                nc.scalar.activation(esc[:, es * 512:(es + 1) * 512], ps,
                                     AF.Identity, bias=bg_s)
            sync.dma_start(out=esc_o[:, it * EPI:(it + 1) * EPI], in_=esc[:NSRT, :])
        if limit == 'esc':
            return
        # final y (block3 output, feature-major; host transposes)
        sync.dma_start(out=yT_o.rearrange("(kt p) n -> p kt n", p=128),
                       in_=yT[:, 0:2, :])


def _item(nc, tc, blk, it, IN_b, O_b, ikt, omt, isq,
          ws, wr, wi, wq, wk, wv, yT, ynm, yinitT, ident,
          xT_h, A_h, a_out,
          ph, pS, pmm, ptr, pA, pxT, phT, pz, pkv, pq, pal, palT, pst, sync):
    """One graph item within one block."""
    c0 = it * SY                  # node col offset of this item
    sl = slice(c0, c0 + SY)

    # y^T source for this block (block1 reads y_init^T)
    def yT_in(kt, cols):
        if blk == 1:
            return yinitT[:, kt, cols]
        return yT[:, kt, cols]

    # ---- load A for this item ----
    A_s = pA.tile([128, 4, NSLT, SY], BF16, tag="A")
    for r in range(NSLT):
        sync.dma_start(out=A_s[:, :, r, :], in_=A_h[it, r].rearrange(
            "(st p) t -> p st t", p=128))

    # ---- message-passing operand ----
    if blk < 3:
        # z_r^T = y^T A_r^T  for all r -> z_s [feat, r, tgt] bf16
        z_s = pz.tile([128, NSLT, ikt, SY], BF16, tag="z")
        for r in range(NSLT):
            for zmt in range(ikt):
                ps = pmm.tile([128, 512], F32, tag="mm")
                for skt in range(4):
                    nc.tensor.matmul(
                        ps,
                        ynm[:, it * 4 + skt, zmt * 128:(zmt + 1) * 128],
                        A_s[:, skt, r, :],
                        start=skt == 0, stop=skt == 3)
                nc.vector.tensor_copy(z_s[:, r, zmt, :], ps)
    else:
        # p_r = y_b @ w_rel[r] (node-major) -> p_s [node, r, feat] bf16
        p_s = pz.tile([128, NSLT, 4, O_b], BF16, tag="z")
        for r in range(NSLT):
            for nmt in range(4):
                ps = pmm.tile([128, O_b], F32, tag="mm")
                for kt in range(ikt):
                    nc.tensor.matmul(
                        ps,
                        _r(yT_in(kt, slice(c0 + nmt * 128, c0 + (nmt + 1) * 128))),
                        _r(wr[:, kt, r, :]),
                        start=kt == 0, stop=kt == ikt - 1)
                nc.vector.tensor_copy(p_s[:, r, nmt, :], ps)

    # ---- h^T accumulation per output M-tile ----
    hT = phT.tile([128, 4, SY], F32R, tag="hT")
    for mt in range(omt):
        mts = slice(mt * 128, (mt + 1) * 128)
        hp = ph.tile([128, SY], F32, tag="h")
        n_mm = ikt + (2 if wi is not None else 0) + \
            (NSLT * ikt if blk < 3 else NSLT * 4)
        i_mm = 0
        for kt in range(ikt):
            nc.tensor.matmul(hp, _r(ws[:, kt, mts]), _r(yT_in(kt, sl)),
                             start=i_mm == 0, stop=i_mm == n_mm - 1)
            i_mm += 1
        if wi is not None:
            for kt in range(2):
                nc.tensor.matmul(hp, _r(wi[:, kt, mts]), _r(yinitT[:, kt, sl]),
                                 start=i_mm == 0, stop=i_mm == n_mm - 1)
                i_mm += 1
        if blk < 3:
            for r in range(NSLT):
                for kt in range(ikt):
                    nc.tensor.matmul(hp, wr[:, kt, r, mts], z_s[:, r, kt, :],
                                     start=i_mm == 0, stop=i_mm == n_mm - 1)
                    i_mm += 1
        else:
            for r in range(NSLT):
                for skt in range(4):
                    nc.tensor.matmul(hp, p_s[:, r, skt, mts], A_s[:, skt, r, :],
                                     start=i_mm == 0, stop=i_mm == n_mm - 1)
                    i_mm += 1
        nc.scalar.copy(hT[:, mt, :], hp)

    # ---- attention ----
    xT_s = pxT.tile([128, 4, SX], BF16, tag="xT")
    sync.dma_start(out=xT_s, in_=xT_h[:, it * SX:(it + 1) * SX].rearrange(
        "(kt p) n -> p kt n", p=128))

    qT = pq.tile([128, 4, SY], BF16, tag="qT")
    for mt in range(omt):
        ps = pmm.tile([128, SY], F32, tag="mm")
        for kt in range(omt):
            nc.tensor.matmul(ps, _r(wq[:, kt, mt * 128:(mt + 1) * 128]),
                             _r(hT[:, kt, :]), start=kt == 0, stop=kt == omt - 1)
        nc.scalar.copy(qT[:, mt, :], ps)

    kT = pkv.tile([128, 4, SX], BF16, tag="kT")
    for mt in range(omt):
        for ns in range(2):
            ps = pmm.tile([128, 512], F32, tag="mm")
            for kt in range(4):
                nc.tensor.matmul(ps, wk[:, kt, mt * 128:(mt + 1) * 128],
                                 xT_s[:, kt, ns * 512:(ns + 1) * 512],
                                 start=kt == 0, stop=kt == 3)
            nc.scalar.copy(kT[:, mt, ns * 512:(ns + 1) * 512], ps)

    v_s = pkv.tile([128, 8, O_b], BF16, tag="v")
    for nmt in range(8):
        ps = pmm.tile([128, O_b], F32, tag="mm")
        for kt in range(4):
            nc.tensor.matmul(ps, xT_s[:, kt, nmt * 128:(nmt + 1) * 128],
                             wv[:, kt, :], start=kt == 0, stop=kt == 3)
        nc.vector.tensor_copy(v_s[:, nmt, :], ps)

    alT = palT.tile([128, 8, SY], BF16, tag="alT")
    for qt in range(4):
        # S in two half-tiles so exp(half) overlaps next half's matmuls
        Sh = []
        mxs = []
        for ns in range(2):
            Sp = pS.tile([128, 512], F32, tag="S")
            for kt in range(omt):
                nc.tensor.matmul(Sp,
                                 qT[:, kt, qt * 128:(qt + 1) * 128],
                                 kT[:, kt, ns * 512:(ns + 1) * 512],
                                 start=kt == 0, stop=kt == omt - 1)
            m = pst.tile([128, 1], F32, tag=f"mx{ns}")
            nc.vector.reduce_max(m, Sp, axis=AX)
            Sh.append(Sp)
            mxs.append(m)
        mx = pst.tile([128, 1], F32, tag="mx")
        nc.vector.tensor_max(mx, mxs[0], mxs[1])
        nm = pst.tile([128, 1], F32, tag="nm")
        nc.vector.tensor_scalar_mul(nm, mx, -isq)
        al = pal.tile([128, SX], F32R, tag="al", bufs=3)
        sums = []
        for ns in range(2):
            s = pst.tile([128, 1], F32, tag=f"s{ns}")
            nc.scalar.activation(al[:, ns * 512:(ns + 1) * 512], Sh[ns],
                                 AF.Exp, bias=nm, scale=isq, accum_out=s)
            sums.append(s)
        ssum = pst.tile([128, 1], F32, tag="ssum")
        nc.vector.tensor_add(ssum, sums[0], sums[1])
        rs = pst.tile([128, 1], F32, tag="rs")
        nc.vector.reciprocal(rs, ssum)
        nc.vector.tensor_scalar_mul(al, al, rs)
        for ds_ in range(2):
            sync.dma_start(
                out=a_out[it, qt * 128:(qt + 1) * 128,
                          ds_ * 512:(ds_ + 1) * 512],
                in_=al[:, ds_ * 512:(ds_ + 1) * 512])
        for kc in range(8):
            tp = ptr.tile([128, 128], F32R, tag="tr")
            nc.tensor.transpose(tp, _r(al[:, kc * 128:(kc + 1) * 128]),
                                _r(ident))
            if kc % 2 == 0:
                nc.vector.tensor_copy(alT[:, kc, qt * 128:(qt + 1) * 128],
                                      tp)
            else:
                nc.scalar.copy(alT[:, kc, qt * 128:(qt + 1) * 128],
                               tp)

    # ctx^T accumulate + y^T = relu(h^T + ctx^T)
    for mt in range(omt):
        cp = ph.tile([128, SY], F32, tag="h")
        for kc in range(8):
            nc.tensor.matmul(cp, v_s[:, kc, mt * 128:(mt + 1) * 128],
                             alT[:, kc, :], start=kc == 0, stop=kc == 7)
        ycols = yT[:, mt, sl]
        nc.vector.tensor_add(ycols, hT[:, mt, :], cp)
        nc.scalar.activation(ycols, ycols, AF.Relu)

    # y node-major for next block's z matmuls (blocks 1,2 only)
    if blk < 3:
        for nt in range(4):
            for fmt in range(omt):
                tp = ptr.tile([128, 128], F32R, tag="tr")
                nc.tensor.transpose(
                    tp, _r(yT[:, fmt, c0 + nt * 128:c0 + (nt + 1) * 128]),
                    _r(ident))
                if fmt % 2 == 0:
                    nc.vector.tensor_copy(
                        ynm[:, it * 4 + nt, fmt * 128:(fmt + 1) * 128],
                        tp)
                else:
                    nc.scalar.copy(
                        ynm[:, it * 4 + nt, fmt * 128:(fmt + 1) * 128],
                        tp)


# ======================= host side =======================

def _host_prep(inputs):
    src_g = np.asarray(inputs["tgt_edge_index"][0]).astype(np.int64)
    tgt_g = np.asarray(inputs["tgt_edge_index"][1]).astype(np.int64)
    et = np.asarray(inputs["tgt_edge_type"]).astype(np.int64)
    tgt_y = np.asarray(inputs["tgt_y"]).astype(np.int64)
    x = np.ascontiguousarray(np.asarray(inputs["x"], dtype=np.float32))

    weights = {}
    for blk in (1, 2, 3):
        weights[f"w{blk}_self"] = np.ascontiguousarray(inputs[f"w{blk}_self"], np.float32)
        rel = np.ascontiguousarray(inputs[f"w{blk}_rel"], np.float32)
        weights[f"w{blk}_rel"] = rel if blk == 3 else rel.astype(BF)
        if blk > 1:
            weights[f"w{blk}_init"] = np.ascontiguousarray(inputs[f"w{blk}_init"], np.float32)
        weights[f"w{blk}_q"] = np.ascontiguousarray(inputs[f"w{blk}_q"], np.float32)
        weights[f"w{blk}_k"] = np.ascontiguousarray(inputs[f"w{blk}_k"], np.float32).astype(BF)
        weights[f"w{blk}_v"] = np.ascontiguousarray(inputs[f"w{blk}_v"], np.float32).astype(BF)
    weights["wz"] = np.ascontiguousarray(inputs["wz"], np.float32)
    wg8 = np.zeros((2 * EMB, 8), np.float32)
    wg8[:, :NSRT] = np.asarray(inputs["wg"], np.float32)
    weights["wg"] = wg8
    weights["bz"] = np.ascontiguousarray(inputs["bz"], np.float32)
    weights["bg"] = np.ascontiguousarray(inputs["bg"], np.float32)
    weights["embed"] = np.ascontiguousarray(inputs["embed"], np.float32)

    in_maps = []
    ar = np.arange(EPI)
    for c in range(NCORES):
        A_T = np.zeros((BPC, NSLT, SY, SY), np.float32)
        GcatT = np.zeros((BPC, 2 * SY, EPI), np.float32)
        for i in range(BPC):
            b = c * BPC + i
            e0 = b * EPI
            s = src_g[e0:e0 + EPI] - b * SY
            t = tgt_g[e0:e0 + EPI] - b * SY
            r = et[e0:e0 + EPI]
            np.add.at(A_T[i], (r, s, t), 1.0)
            GcatT[i][s, ar] += 1.0
            GcatT[i][SY + t, ar] += 1.0
        ty = tgt_y[c * NYC:(c + 1) * NYC]
        onehotT = np.zeros((V, NYC), np.float32)
        onehotT[ty, np.arange(NYC)] = 1.0
        xT = np.ascontiguousarray(x[c * NXC:(c + 1) * NXC].T).astype(BF)
        m = dict(weights)
        m["xT"] = xT
        m["A_T"] = A_T.astype(BF)
        m["GcatT"] = GcatT.astype(BF)
        m["onehotT"] = onehotT
        m["ident"] = np.eye(128, dtype=np.float32)
        in_maps.append(m)
    return in_maps


def _get_nc(limit=None):
    key = ("nc", limit)
    if key not in _CACHE:
        _CACHE[key] = _build_program(limit)
    return _CACHE[key]


def kernel(**inputs):
    nc = _get_nc(_CACHE.get('limit'))
    in_maps = _host_prep(inputs)
    res = run_bass_kernel_spmd(nc, in_maps, core_ids=list(range(NCORES)),
                               **_CACHE.get("run_kwargs", {}))
    _CACHE["last_results"] = res
    rs = res.results
    y = np.concatenate([np.ascontiguousarray(r["yT_out"].T) for r in rs], 0)
    y_score = np.concatenate([np.ascontiguousarray(r["yscoreT_out"].T) for r in rs], 0)
    y_edge = np.concatenate([np.ascontiguousarray(r["escoreT_out"].T) for r in rs], 0)
    a1 = np.concatenate([r["a1_out"] for r in rs], 0)
    a2 = np.concatenate([r["a2_out"] for r in rs], 0)
    a3 = np.concatenate([r["a3_out"] for r in rs], 0)
    return (y, inputs["tgt_y_batch"], inputs["tgt_edge_index"],
            inputs["tgt_edge_type"], y_score, y_edge, a1, a2, a3)
